# revision 1
# baseline (speedup 1.0000x reference)
"""DEQ transformer block with Anderson acceleration on 8 Trainium2 NeuronCores.

v2: fp16 matmuls, persistent attention weights in SBUF, streamed MLP weights,
token-major Anderson state with fused scalar_tensor_tensor updates.

Sharding: each of the 4 sequences (B=4) is split across a pair of cores
(512 tokens each).  K/V are exchanged within each pair via AllGather every
DEQ iteration.  Matmul activations are fp16; the residual stream (z), the
Anderson Gram solve and its coefficients stay fp32.  LayerNorm weight/bias
are folded into the following projection weights host-side.
"""

import numpy as np

P = 128
TL = 512          # tokens per core (half a sequence)
C = 768
CCN = 6           # C / 128
TCH = 4           # token chunks of 128
NH = 12
DH = 64
HPN = 6           # head pairs: chunk j holds head j (rows 0:64), j+6 (64:128)
NHID = 3072
HCN = 24          # NHID / 128
KCN = 8           # full-seq key chunks (1024 / 128)
VW = 64           # per-head V width
VA = NH * VW      # 768
MH = 5            # Anderson history slots
LN_EPS = 1e-5
NCORES = 8
GROUPS = [[0, 1], [2, 3], [4, 5], [6, 7]]

_CACHE = {}


def _build(num_iters):
    from contextlib import ExitStack
    import concourse.bass as bass  # noqa
    import concourse.mybir as mybir
    import concourse.tile as tile
    from concourse import bacc
    from concourse.masks import make_identity

    FP = mybir.dt.float32
    H = mybir.dt.float16
    AF = mybir.ActivationFunctionType
    OP = mybir.AluOpType

    nc = bacc.Bacc()
    ni = num_iters

    # ---------------- DRAM I/O ----------------
    uit_d = nc.dram_tensor("u_it", [ni, TL, C], H, kind="ExternalInput")
    qkw_d = nc.dram_tensor("qkw_pack", [P, 12, CCN, P], H, kind="ExternalInput")
    vw_d = nc.dram_tensor("vw_pack", [P, CCN, VA], H, kind="ExternalInput")
    wo_d = nc.dram_tensor("wo_pack", [P, CCN, CCN, P], H, kind="ExternalInput")
    w1_d = nc.dram_tensor("w1_pack", [HCN, P, CCN, P], H, kind="ExternalInput")
    w2_d = nc.dram_tensor("w2_pack", [CCN, 2, P, 12, P], H, kind="ExternalInput")
    vb_d = nc.dram_tensor("vb_aug", [1, VA], H, kind="ExternalInput")
    bqk_d = nc.dram_tensor("bqk_cols", [P, 12], FP, kind="ExternalInput")
    bo_d = nc.dram_tensor("bo_cols", [P, CCN], FP, kind="ExternalInput")
    b1_d = nc.dram_tensor("b1_cols", [P, HCN], FP, kind="ExternalInput")
    b2_d = nc.dram_tensor("b2_cols", [P, CCN], FP, kind="ExternalInput")
    zo_d = nc.dram_tensor("z_out", [TL, C], FP, kind="ExternalOutput")

    # internal DRAM
    kcc = nc.dram_tensor("k_cc", [C, TL], H)
    vcc = nc.dram_tensor("v_cc", [TL, VA], H)
    kall = nc.dram_tensor("k_all", [2, C, TL], H)
    vall = nc.dram_tensor("v_all", [2, TL, VA], H)
    fh = nc.dram_tensor("f_hist", [MH, TL, C], H)

    with tile.TileContext(nc) as tc:
        ctx = ExitStack()
        pers = ctx.enter_context(tc.tile_pool(name="pers", bufs=1))
        uitp = ctx.enter_context(tc.tile_pool(name="uitp", bufs=2))
        big16 = ctx.enter_context(tc.tile_pool(name="big16", bufs=2))
        fm16 = ctx.enter_context(tc.tile_pool(name="fm16", bufs=1))
        kp = ctx.enter_context(tc.tile_pool(name="kp", bufs=1))
        vp = ctx.enter_context(tc.tile_pool(name="vp", bufs=1))
        qp = ctx.enter_context(tc.tile_pool(name="qp", bufs=1))
        ofm = ctx.enter_context(tc.tile_pool(name="ofm", bufs=1))
        atp = ctx.enter_context(tc.tile_pool(name="atp", bufs=1))
        tokp = ctx.enter_context(tc.tile_pool(name="tokp", bufs=2))
        gp = ctx.enter_context(tc.tile_pool(name="gp", bufs=HCN))
        w1s = ctx.enter_context(tc.tile_pool(name="w1s", bufs=3))
        w2s = ctx.enter_context(tc.tile_pool(name="w2s", bufs=2))
        fhs = ctx.enter_context(tc.tile_pool(name="fhs", bufs=5))
        dfp = ctx.enter_context(tc.tile_pool(name="dfp", bufs=4))
        vec = ctx.enter_context(tc.tile_pool(name="vec", bufs=6))
        vsp = ctx.enter_context(tc.tile_pool(name="vsp", bufs=2))
        rrp = ctx.enter_context(tc.tile_pool(name="rrp", bufs=2))
        jk = ctx.enter_context(tc.tile_pool(name="jk", bufs=3))
        pA = ctx.enter_context(tc.tile_pool(name="pA", bufs=2, space="PSUM"))
        pB = ctx.enter_context(tc.tile_pool(name="pB", bufs=2, space="PSUM"))
        pC = ctx.enter_context(tc.tile_pool(name="pC", bufs=2, space="PSUM"))
        pD = ctx.enter_context(tc.tile_pool(name="pD", bufs=2, space="PSUM"))

        # ------------- persistent tiles -------------
        qkw_sb = pers.tile([P, 12, CCN, P], H, name="qkw_sb")
        vw_sb = pers.tile([P, CCN, VA], H, name="vw_sb")
        wo_sb = pers.tile([P, CCN, CCN, P], H, name="wo_sb")
        bqk_sb = pers.tile([P, 12], FP, name="bqk_sb")
        bo_sb = pers.tile([P, CCN], FP, name="bo_sb")
        b1_sb = pers.tile([P, HCN], FP, name="b1_sb")
        b2_sb = pers.tile([P, CCN], FP, name="b2_sb")
        vb_sb = pers.tile([1, VA], H, name="vb_sb")
        ident16 = pers.tile([P, P], H, name="ident16")
        ones_sb = pers.tile([P, P], H, name="ones_sb")
        z_sb = pers.tile([P, TCH, C], FP, name="z_sb")
        stat = pers.tile([P, 8, TCH], FP, name="stat")
        dots_f = pers.tile([P, 14 * TCH], FP, name="dots_f")
        work = pers.tile([P, TCH, 20], FP, name="work")
        coef = pers.tile([P, TCH, MH], FP, name="coef")
        eps_col = pers.tile([P, 1], FP, name="eps_col")
        sc1 = pers.tile([P, TCH], FP, name="sc1")
        sc2 = pers.tile([P, TCH], FP, name="sc2")
        sc3 = pers.tile([P, TCH], FP, name="sc3")
        alt = pers.tile([P, TCH, 4], FP, name="alt")
        tb = pers.tile([64, CCN, TL], H, name="tb")

        nc.sync.dma_start(qkw_sb[:], qkw_d[:])
        nc.sync.dma_start(vw_sb[:], vw_d[:])
        nc.sync.dma_start(wo_sb[:], wo_d[:])
        nc.sync.dma_start(bqk_sb[:], bqk_d[:])
        nc.sync.dma_start(bo_sb[:], bo_d[:])
        nc.sync.dma_start(b1_sb[:], b1_d[:])
        nc.sync.dma_start(b2_sb[:], b2_d[:])
        nc.sync.dma_start(vb_sb[:], vb_d[:])
        make_identity(nc, ident16[:])
        nc.vector.memset(ones_sb[:], 1.0)
        nc.vector.memset(eps_col[:], LN_EPS)

        TT = nc.vector.tensor_tensor
        TS = nc.vector.tensor_scalar
        STT = nc.vector.scalar_tensor_tensor
        TTR = nc.vector.tensor_tensor_reduce

        def layernorm(src, dst, sc):
            """token-major LN without weight/bias (folded into next matmul).
            src/dst: [P, TCH, C] fp16; stats use stat cols 4*sc..4*sc+3."""
            i0, i1, i2, i3 = 4 * sc, 4 * sc + 1, 4 * sc + 2, 4 * sc + 3
            for t in range(TCH):
                j1 = jk.tile([P, C], H, name="jk")
                j2 = jk.tile([P, C], H, name="jk")
                nc.scalar.activation(j1[:], src[:, t], AF.Identity,
                                     accum_out=stat[:, i0, t:t + 1])
                nc.scalar.activation(j2[:], src[:, t], AF.Square,
                                     accum_out=stat[:, i1, t:t + 1])
            TS(out=stat[:, i2], in0=stat[:, i0], scalar1=1.0 / C,
               scalar2=None, op0=OP.mult)                       # mu
            TT(out=stat[:, i0], in0=stat[:, i2], in1=stat[:, i2],
               op=OP.mult)                                      # mu^2
            TS(out=stat[:, i1], in0=stat[:, i1], scalar1=1.0 / C,
               scalar2=None, op0=OP.mult)                       # E[x^2]
            TT(out=stat[:, i1], in0=stat[:, i1], in1=stat[:, i0],
               op=OP.subtract)                                  # var
            nc.scalar.activation(stat[:, i0], stat[:, i1], AF.Sqrt,
                                 bias=eps_col[:])               # sd
            nc.vector.reciprocal_approx_fast(stat[:, i3], stat[:, i0])
            for t in range(TCH):
                with nc.allow_low_precision(reason="fp16 ln out"):
                    TS(out=dst[:, t], in0=src[:, t],
                       scalar1=stat[:, i2, t:t + 1],
                       scalar2=stat[:, i3, t:t + 1],
                       op0=OP.subtract, op1=OP.mult)

        def transpose_fm(src_tok, dst_fm):
            """[P, TCH, C] fp16 token-major -> [P, CCN, TL] feature-major."""
            for cc in range(CCN):
                ptr = pA.tile([P, TCH, P], H, name="pA")
                for t in range(TCH):
                    nc.tensor.transpose(
                        ptr[:, t], src_tok[:, t, cc * P:(cc + 1) * P],
                        ident16[:])
                nc.scalar.copy(dst_fm[:, cc], ptr[:])

        def transpose_tok(src_fm, dst_tok):
            """[P, CCN, TL] fp16 feature-major -> [P, TCH, C] token-major."""
            for t in range(TCH):
                ptr = pA.tile([P, CCN, P], H, name="pA")
                for cc in range(CCN):
                    nc.tensor.transpose(
                        ptr[:, cc], src_fm[:, cc, t * P:(t + 1) * P],
                        ident16[:])
                nc.scalar.copy(dst_tok[:, t], ptr[:])

        for it in range(ni):
            Kn = min(it, 4)
            s_new = it % MH
            prev = [(it - Kn + k) % MH for k in range(Kn)]  # oldest..newest

            # ---- A: zctx = z + (u + 0.1 emb_it); LN1 -> x1 ----
            zctx = big16.tile([P, TCH, C], H, name="big16")
            for t in range(TCH):
                ut = uitp.tile([P, C], H, name="uitp")
                nc.sync.dma_start(ut[:], uit_d[it, t * P:(t + 1) * P, :])
                if it == 0:
                    nc.vector.tensor_copy(zctx[:, t], ut[:])
                else:
                    with nc.allow_low_precision(reason="fp16 zctx"):
                        TT(out=zctx[:, t], in0=z_sb[:, t], in1=ut[:],
                           op=OP.add)
            x1t = big16.tile([P, TCH, C], H, name="big16")
            layernorm(zctx, x1t, 0)
            x1 = fm16.tile([P, CCN, TL], H, name="fm16")
            transpose_fm(x1t, x1)

            # ---- B: K projection -> send ----
            for oc in range(CCN):
                pk = pB.tile([P, TL], FP, name="pB")
                for cc in range(CCN):
                    nc.tensor.matmul(pk[:], qkw_sb[:, 6 + oc, cc], x1[:, cc],
                                     start=(cc == 0), stop=(cc == CCN - 1))
                kst = vec.tile([P, TL], H, name="v16")
                nc.scalar.activation(kst[:], pk[:], AF.Identity,
                                     bias=bqk_sb[:, 6 + oc:7 + oc])
                nc.sync.dma_start(kcc[oc * P:(oc + 1) * P, :], kst[:])
            nc.gpsimd.collective_compute(
                "AllGather", OP.bypass, replica_groups=GROUPS,
                ins=[kcc[:]], outs=[kall[:]])

            # ---- C: V projection (token-major, aug ones col) -> send ----
            for t in range(TCH):
                pva = pB.tile([P, TL], FP, name="pB")
                pvb = pD.tile([P, VA - TL], FP, name="pD")
                for cc in range(CCN):
                    nc.tensor.matmul(pva[:], x1[:, cc, t * P:(t + 1) * P],
                                     vw_sb[:, cc, 0:TL],
                                     start=(cc == 0), stop=False)
                    nc.tensor.matmul(pvb[:], x1[:, cc, t * P:(t + 1) * P],
                                     vw_sb[:, cc, TL:VA],
                                     start=(cc == 0), stop=False)
                nc.tensor.matmul(pva[:], ones_sb[0:1, :], vb_sb[:, 0:TL],
                                 start=False, stop=True)
                nc.tensor.matmul(pvb[:], ones_sb[0:1, :], vb_sb[:, TL:VA],
                                 start=False, stop=True)
                vst = vsp.tile([P, VA], H, name="vsp")
                nc.scalar.copy(vst[:, 0:TL], pva[:])
                nc.scalar.copy(vst[:, TL:VA], pvb[:])
                nc.sync.dma_start(vcc[t * P:(t + 1) * P, :], vst[:])
            nc.gpsimd.collective_compute(
                "AllGather", OP.bypass, replica_groups=GROUPS,
                ins=[vcc[:]], outs=[vall[:]])

            # ---- D: Q projection (overlaps collectives) ----
            q_sb = qp.tile([P, CCN, TL], H, name="q_sb")
            for oc in range(CCN):
                pq = pB.tile([P, TL], FP, name="pB")
                for cc in range(CCN):
                    nc.tensor.matmul(pq[:], qkw_sb[:, oc, cc], x1[:, cc],
                                     start=(cc == 0), stop=(cc == CCN - 1))
                nc.scalar.activation(q_sb[:, oc], pq[:], AF.Identity,
                                     bias=bqk_sb[:, oc:oc + 1])

            # ---- E: read back gathered K/V ----
            k_sb = kp.tile([P, CCN, 2 * TL], H, name="k_sb")
            v_sb = vp.tile([P, KCN, VA], H, name="v_sb")
            for r in range(2):
                for cc in range(CCN):
                    nc.sync.dma_start(
                        k_sb[:, cc, r * TL:(r + 1) * TL],
                        kall[r, cc * P:(cc + 1) * P, :])
                for t in range(TCH):
                    nc.sync.dma_start(v_sb[:, r * TCH + t],
                                      vall[r, t * P:(t + 1) * P, :])

            # ---- F: attention ----
            out_fm = ofm.tile([P, CCN, TL], H, name="ofm")
            for hp in range(HPN):
                ha, hb = hp, hp + 6
                pava = pC.tile([64, TL], FP, name="pC")
                pavb = pC.tile([64, TL], FP, name="pC")
                psa = pD.tile([64, TL], FP, name="pD")
                psb = pD.tile([64, TL], FP, name="pD")
                for kc in range(KCN):
                    ks = slice(kc * P, (kc + 1) * P)
                    sca = pB.tile([P, TL], FP, name="pB")
                    scb = pB.tile([P, TL], FP, name="pB")
                    nc.tensor.matmul(sca[:], k_sb[0:64, hp, ks],
                                     q_sb[0:64, hp], start=True, stop=True)
                    nc.tensor.matmul(scb[:], k_sb[64:128, hp, ks],
                                     q_sb[64:128, hp], start=True, stop=True)
                    atta = vec.tile([P, TL], H, name="v16")
                    attb = vec.tile([P, TL], H, name="v16")
                    nc.scalar.activation(atta[:], sca[:], AF.Exp, scale=0.125)
                    nc.scalar.activation(attb[:], scb[:], AF.Exp, scale=0.125)
                    nc.tensor.matmul(pava[:],
                                     v_sb[:, kc, ha * VW:(ha + 1) * VW],
                                     atta[:], start=(kc == 0),
                                     stop=(kc == KCN - 1))
                    nc.tensor.matmul(pavb[:],
                                     v_sb[:, kc, hb * VW:(hb + 1) * VW],
                                     attb[:], start=(kc == 0),
                                     stop=(kc == KCN - 1))
                    nc.tensor.matmul(psa[:], ones_sb[:, 0:64], atta[:],
                                     start=(kc == 0), stop=(kc == KCN - 1))
                    nc.tensor.matmul(psb[:], ones_sb[:, 0:64], attb[:],
                                     start=(kc == 0), stop=(kc == KCN - 1))
                ra = rrp.tile([64, TL], FP, name="rrp")
                rb = rrp.tile([64, TL], FP, name="rrp")
                nc.vector.reciprocal_approx_fast(ra[:], psa[:])
                nc.vector.reciprocal_approx_fast(rb[:], psb[:])
                with nc.allow_low_precision(reason="fp16 attn"):
                    TT(out=out_fm[0:64, hp], in0=pava[0:64, :], in1=ra[:],
                       op=OP.mult)
                    TT(out=tb[:, hp], in0=pavb[0:64, :], in1=rb[:],
                       op=OP.mult)
            nc.sync.dma_start(out_fm[64:128, :, :], tb[:])

            # ---- G: output projection -> attn (feature-major fp16) ----
            attn_fm = atp.tile([P, CCN, TL], H, name="atp")
            for oc in range(CCN):
                pp_ = pA.tile([P, TL], FP, name="pA")
                for ci in range(CCN):
                    nc.tensor.matmul(pp_[:], wo_sb[:, oc, ci], out_fm[:, ci],
                                     start=(ci == 0), stop=(ci == CCN - 1))
                nc.scalar.activation(attn_fm[:, oc], pp_[:], AF.Identity,
                                     bias=bo_sb[:, oc:oc + 1])

            # ---- H: za = z + attn (token-major); LN2 -> x2 ----
            attn_tok = tokp.tile([P, TCH, C], H, name="tokp")
            transpose_tok(attn_fm, attn_tok)
            if it == 0:
                za = attn_tok
            else:
                za = big16.tile([P, TCH, C], H, name="big16")
                for t in range(TCH):
                    with nc.allow_low_precision(reason="fp16 za"):
                        TT(out=za[:, t], in0=z_sb[:, t], in1=attn_tok[:, t],
                           op=OP.add)
            x2t = big16.tile([P, TCH, C], H, name="big16")
            layernorm(za, x2t, 1)
            x2 = fm16.tile([P, CCN, TL], H, name="fm16")
            transpose_fm(x2t, x2)

            # ---- I: MLP (feature-major, streamed weights) ----
            gts = []
            for hi in range(HCN):
                w1t = w1s.tile([P, CCN, P], H, name="w1s")
                nc.sync.dma_start(w1t[:], w1_d[hi])
                ph = pA.tile([P, TL], FP, name="pA")
                for cc in range(CCN):
                    nc.tensor.matmul(ph[:], w1t[:, cc], x2[:, cc],
                                     start=(cc == 0), stop=(cc == CCN - 1))
                gt = gp.tile([P, TL], H, name="gp")
                nc.scalar.activation(gt[:], ph[:], AF.Gelu,
                                     bias=b1_sb[:, hi:hi + 1])
                gts.append(gt)
            res_fm = ofm.tile([P, CCN, TL], H, name="ofm")
            for oc in range(CCN):
                po = pB.tile([P, TL], FP, name="pB")
                for half in range(2):
                    w2t = w2s.tile([P, 12, P], H, name="w2s")
                    nc.sync.dma_start(w2t[:], w2_d[oc, half])
                    for hj in range(12):
                        hi = half * 12 + hj
                        nc.tensor.matmul(po[:], w2t[:, hj], gts[hi][:],
                                         start=(hi == 0), stop=(hi == HCN - 1))
                # res = (po + b2) + attn
                with nc.allow_low_precision(reason="fp16 res"):
                    STT(out=res_fm[:, oc], in0=po[:],
                        scalar=b2_sb[:, oc:oc + 1], in1=attn_fm[:, oc],
                        op0=OP.add, op1=OP.add)

            # ---- J: res -> token-major, store history ----
            resq = tokp.tile([P, TCH, C], H, name="tokp")
            transpose_tok(res_fm, resq)
            for t in range(TCH):
                nc.sync.dma_start(fh[s_new, t * P:(t + 1) * P, :], resq[:, t])

            # ---- K: Anderson update ----
            if Kn == 0:
                for t in range(TCH):
                    nc.vector.tensor_copy(z_sb[:, t], resq[:, t])
            else:
                pairs = [(a, b) for a in range(Kn) for b in range(a, Kn)]
                pairs += [(k, Kn) for k in range(Kn)]
                np_ = len(pairs)
                pidx = {p: i for i, p in enumerate(pairs)}
                for t in range(TCH):
                    dfs = []
                    for k in range(Kn):
                        ft = fhs.tile([P, C], H, name="fhs")
                        nc.sync.dma_start(ft[:],
                                          fh[prev[k], t * P:(t + 1) * P, :])
                        df = dfp.tile([P, C], H, name="dfp")
                        with nc.allow_low_precision(reason="fp16 df"):
                            TT(out=df[:], in0=ft[:], in1=resq[:, t],
                               op=OP.subtract)
                        dfs.append(df)
                    for pi, (a, b) in enumerate(pairs):
                        ina = dfs[a][:]
                        inb = dfs[b][:] if b < Kn else resq[:, t]
                        j1 = jk.tile([P, C], H, name="jk")
                        with nc.allow_low_precision(reason="junk out"):
                            STT(out=j1[:], in0=ina, scalar=1.0, in1=inb,
                                op0=OP.mult, op1=OP.mult,
                                accum_out=dots_f[:, pi * TCH + t:
                                                 pi * TCH + t + 1])

                # expand symmetric dots into full KxK G (cols 0..15) and
                # b-vector (cols 16..19) -- elimination needs full storage
                def As(a, b):
                    return work[:, :, a * 4 + b]

                def Bs(k):
                    return work[:, :, 16 + k]

                for a in range(Kn):
                    for b in range(Kn):
                        p = pidx[(min(a, b), max(a, b))]
                        nc.vector.tensor_copy(
                            As(a, b), dots_f[:, p * TCH:(p + 1) * TCH])
                    TS(out=As(a, a), in0=As(a, a), scalar1=1e-6,
                       scalar2=None, op0=OP.add)
                    p = pidx[(a, Kn)]
                    nc.vector.tensor_copy(
                        Bs(a), dots_f[:, p * TCH:(p + 1) * TCH])

                for i in range(Kn):
                    nc.vector.reciprocal(sc1[:], As(i, i))
                    for j in range(i + 1, Kn):
                        TT(out=sc2[:], in0=As(j, i), in1=sc1[:], op=OP.mult)
                        for m in range(i, Kn):
                            TT(out=sc3[:], in0=sc2[:], in1=As(i, m),
                               op=OP.mult)
                            TT(out=As(j, m), in0=As(j, m), in1=sc3[:],
                               op=OP.subtract)
                        TT(out=sc3[:], in0=sc2[:], in1=Bs(i), op=OP.mult)
                        TT(out=Bs(j), in0=Bs(j), in1=sc3[:], op=OP.subtract)
                for i in range(Kn - 1, -1, -1):
                    nc.vector.tensor_copy(sc3[:], Bs(i))
                    for j in range(i + 1, Kn):
                        TT(out=sc2[:], in0=As(i, j), in1=alt[:, :, j],
                           op=OP.mult)
                        TT(out=sc3[:], in0=sc3[:], in1=sc2[:], op=OP.subtract)
                    nc.vector.reciprocal(sc1[:], As(i, i))
                    TT(out=alt[:, :, i], in0=sc3[:], in1=sc1[:], op=OP.mult)

                # coef col 0 = 1 + sum(alpha); cols 1..Kn = -alpha
                if Kn == 1:
                    TS(out=coef[:, :, 0], in0=alt[:, :, 0], scalar1=1.0,
                       scalar2=None, op0=OP.add)
                else:
                    nc.vector.tensor_copy(sc1[:], alt[:, :, 0])
                    for k in range(1, Kn):
                        TT(out=sc1[:], in0=sc1[:], in1=alt[:, :, k],
                           op=OP.add)
                    TS(out=coef[:, :, 0], in0=sc1[:], scalar1=1.0,
                       scalar2=None, op0=OP.add)
                TS(out=coef[:, :, 1:1 + Kn], in0=alt[:, :, 0:Kn],
                   scalar1=-1.0, scalar2=None, op0=OP.mult)

                # z += c0*res + sum_k c_{k+1}*F_k   (fused STT per term)
                for t in range(TCH):
                    STT(out=z_sb[:, t], in0=resq[:, t],
                        scalar=coef[:, t, 0:1], in1=z_sb[:, t],
                        op0=OP.mult, op1=OP.add)
                    for k in range(Kn):
                        ft = fhs.tile([P, C], H, name="fhs")
                        nc.sync.dma_start(ft[:],
                                          fh[prev[k], t * P:(t + 1) * P, :])
                        STT(out=z_sb[:, t], in0=ft[:],
                            scalar=coef[:, t, k + 1:k + 2], in1=z_sb[:, t],
                            op0=OP.mult, op1=OP.add)

        for t in range(TCH):
            nc.sync.dma_start(zo_d[t * P:(t + 1) * P, :], z_sb[:, t])

        ctx.close()

    nc.finalize()
    return nc


def _host_pack(inputs, num_iters):
    f32 = np.float32
    f16 = np.float16
    ipw = np.ascontiguousarray(inputs["in_proj_w"], f32)
    ipb = np.ascontiguousarray(inputs["in_proj_b"], f32)
    opw = np.ascontiguousarray(inputs["out_proj_w"], f32)
    opb = np.ascontiguousarray(inputs["out_proj_b"], f32)
    w1 = np.ascontiguousarray(inputs["mlp_w1"], f32)
    b1 = np.ascontiguousarray(inputs["mlp_b1"], f32)
    w2 = np.ascontiguousarray(inputs["mlp_w2"], f32)
    b2 = np.ascontiguousarray(inputs["mlp_b2"], f32)
    emb = np.ascontiguousarray(inputs["iter_emb"], f32)
    ln1_w = np.asarray(inputs["ln1_w"], f32)
    ln1_b = np.asarray(inputs["ln1_b"], f32)
    ln2_w = np.asarray(inputs["ln2_w"], f32)
    ln2_b = np.asarray(inputs["ln2_b"], f32)

    # fold LN1 into in_proj, LN2 into mlp_w1
    ipw_f = ipw * ln1_w[None, :]
    ipb_f = ipb + ipw @ ln1_b
    w1_f = w1 * ln2_w[None, :]
    b1_f = b1 + w1 @ ln2_b

    # head permutation: attn chunk j holds head j (rows 0:64), head j+6
    # (rows 64:128)
    hperm = np.zeros(C, np.int64)
    for j in range(HPN):
        hperm[j * P:j * P + 64] = np.arange(j * 64, (j + 1) * 64)
        hperm[j * P + 64:(j + 1) * P] = np.arange((j + 6) * 64, (j + 7) * 64)

    qw = ipw_f[0:C][hperm]
    kw = ipw_f[C:2 * C][hperm]
    vw = ipw_f[2 * C:3 * C]
    qb = ipb_f[0:C][hperm]
    kb = ipb_f[C:2 * C][hperm]
    vb = ipb_f[2 * C:3 * C]

    # qkw_pack [P(c within chunk), 12, CCN, P(m)]: chunks 0..5 q, 6..11 k
    qkw = np.concatenate([qw.reshape(CCN, P, C), kw.reshape(CCN, P, C)], 0)
    qkw_pack = np.ascontiguousarray(
        qkw.reshape(12, P, CCN, P).transpose(3, 0, 2, 1).astype(f16))

    # vw_pack [P(c), CCN, VA] (plain v feature order)
    vw_aug = vw.T.astype(f32)
    vb_aug = vb.astype(f32)
    vw_pack = np.ascontiguousarray(
        vw_aug.reshape(CCN, P, VA).transpose(1, 0, 2).astype(f16))

    # wo_pack [P(c-attnfeat), oc, ci, P(m)] (columns permuted by hperm)
    opw_p = opw[:, hperm]
    wo_pack = np.ascontiguousarray(
        opw_p.reshape(CCN, P, CCN, P).transpose(3, 0, 2, 1).astype(f16))

    # w1_pack [hi, P(c), cc, P(m)]
    w1_pack = np.ascontiguousarray(
        w1_f.reshape(HCN, P, CCN, P).transpose(0, 3, 2, 1).astype(f16))

    # w2_pack [oc, half, P(hid), hj, P(m)]
    w2_pack = np.ascontiguousarray(
        w2.reshape(CCN, P, 2, 12, P).transpose(0, 2, 4, 3, 1).astype(f16))

    bqk_cols = np.ascontiguousarray(
        np.concatenate([qb, kb]).reshape(12, P).T.astype(f32))
    bo_cols = np.ascontiguousarray(opb.reshape(CCN, P).T.astype(f32))
    b1_cols = np.ascontiguousarray(b1_f.reshape(HCN, P).T.astype(f32))
    b2_cols = np.ascontiguousarray(b2.reshape(CCN, P).T.astype(f32))

    rows = [min(i, emb.shape[0] - 1) for i in range(num_iters)]
    u = np.ascontiguousarray(inputs["u"], f32)

    shared = dict(
        qkw_pack=qkw_pack, vw_pack=vw_pack, wo_pack=wo_pack, w1_pack=w1_pack,
        w2_pack=w2_pack, vb_aug=vb_aug.reshape(1, VA).astype(f16),
        bqk_cols=bqk_cols, bo_cols=bo_cols, b1_cols=b1_cols, b2_cols=b2_cols)
    in_maps = []
    for core in range(NCORES):
        b, h = core // 2, core % 2
        m = dict(shared)
        useg = u[b, h * TL:(h + 1) * TL, :]
        u_it = useg[None] + 0.1 * emb[rows][:, None, :]
        m["u_it"] = np.ascontiguousarray(u_it.astype(f16))
        in_maps.append(m)
    return in_maps


def run_device(inputs, num_iters=None, trace=False):
    from concourse.bass_utils import run_bass_kernel_spmd
    ni = int(inputs.get("num_iters", 6)) if num_iters is None else num_iters
    u = inputs["u"]
    B, T, _ = u.shape
    if ni == 0:
        return np.zeros((B, T, C), np.float32), None
    if ni not in _CACHE:
        _CACHE[ni] = _build(ni)
    nc = _CACHE[ni]
    in_maps = _host_pack(inputs, ni)
    r = run_bass_kernel_spmd(nc, in_maps, list(range(NCORES)), trace=trace)
    out = np.empty((B, T, C), np.float32)
    for core in range(NCORES):
        b, h = core // 2, core % 2
        out[b, h * TL:(h + 1) * TL, :] = r.results[core]["z_out"]
    return out, r


def kernel(**inputs):
    out, _ = run_device(inputs)
    return out.astype(np.float32)



# revision 6
# speedup vs baseline: 1.0501x; 1.0501x over previous
"""DEQ transformer block with Anderson acceleration on 8 Trainium2 NeuronCores.

v3: single activation-table set for LN (ln+exp rsqrt), exp batched over 2 PSUM
banks, AllReduce-based K/V exchange (remote = sum - local) so local-half
attention starts before the collective lands, Gram-matrix caching for the
Anderson least squares (only Kn+1 new dot products per iteration), row-batched
Gaussian elimination, residual history held in SBUF, gpsimd/vector split for
the z update, and PE keep-alive transposes through the Anderson phase.

Sharding: each of the 4 sequences (B=4) is split across a pair of cores
(512 tokens each).  K/V are exchanged within each pair via AllReduce every
DEQ iteration.  Matmul activations are fp16; the residual stream (z), the
Anderson Gram solve and its coefficients stay fp32.  LayerNorm weight/bias
are folded into the following projection weights host-side.
"""

import numpy as np

P = 128
TL = 512          # tokens per core (half a sequence)
C = 768
CCN = 6           # C / 128
TCH = 4           # token chunks of 128
NH = 12
DH = 64
HPN = 6           # head pairs: chunk j holds head j (rows 0:64), j+6 (64:128)
NHID = 3072
HCN = 24          # NHID / 128
KCN = 8           # full-seq key chunks (1024 / 128)
VW = 64           # per-head V width
VA = NH * VW      # 768
MH = 5            # Anderson history slots
LN_EPS = 1e-5
NCORES = 8
GROUPS = [[0, 1], [2, 3], [4, 5], [6, 7]]

_CACHE = {}


def _build(num_iters):
    from contextlib import ExitStack
    import concourse.bass as bass  # noqa
    import concourse.mybir as mybir
    import concourse.tile as tile
    from concourse import bacc
    from concourse.masks import make_identity

    FP = mybir.dt.float32
    H = mybir.dt.float16
    AF = mybir.ActivationFunctionType
    OP = mybir.AluOpType

    nc = bacc.Bacc()
    ni = num_iters

    # ---------------- DRAM I/O ----------------
    uit_d = nc.dram_tensor("u_it", [ni, TL, C], H, kind="ExternalInput")
    qkw_d = nc.dram_tensor("qkw_pack", [P, 12, CCN, P], H, kind="ExternalInput")
    vw_d = nc.dram_tensor("vw_pack", [P, CCN, VA], H, kind="ExternalInput")
    wo_d = nc.dram_tensor("wo_pack", [P, CCN, CCN, P], H, kind="ExternalInput")
    w1_d = nc.dram_tensor("w1_pack", [HCN, P, CCN, P], H, kind="ExternalInput")
    w2_d = nc.dram_tensor("w2_pack", [HCN, P, CCN, P], H, kind="ExternalInput")
    vb_d = nc.dram_tensor("vb_aug", [1, VA], H, kind="ExternalInput")
    bqk_d = nc.dram_tensor("bqk_cols", [P, 12], FP, kind="ExternalInput")
    bo_d = nc.dram_tensor("bo_cols", [P, CCN], FP, kind="ExternalInput")
    b1_d = nc.dram_tensor("b1_cols", [P, HCN], FP, kind="ExternalInput")
    b2_d = nc.dram_tensor("b2_cols", [P, CCN], FP, kind="ExternalInput")
    zo_d = nc.dram_tensor("z_out", [TL, C], FP, kind="ExternalOutput")

    # internal DRAM (collective staging)
    kcc = nc.dram_tensor("k_cc", [C, TL], H)
    vcc = nc.dram_tensor("v_cc", [TL, VA], H)
    kred = nc.dram_tensor("k_red", [C, TL], H)
    vred = nc.dram_tensor("v_red", [TL, VA], H)

    with tile.TileContext(nc) as tc:
        ctx = ExitStack()
        pers = ctx.enter_context(tc.tile_pool(name="pers", bufs=1))
        uitp = ctx.enter_context(tc.tile_pool(name="uitp", bufs=2))
        big16 = ctx.enter_context(tc.tile_pool(name="big16", bufs=2))
        fm16 = ctx.enter_context(tc.tile_pool(name="fm16", bufs=2))
        qp = ctx.enter_context(tc.tile_pool(name="qp", bufs=1))
        ofm = ctx.enter_context(tc.tile_pool(name="ofm", bufs=2))
        atp = ctx.enter_context(tc.tile_pool(name="atp", bufs=1))
        tokp = ctx.enter_context(tc.tile_pool(name="tokp", bufs=1))
        gp = ctx.enter_context(tc.tile_pool(name="gp", bufs=3))
        w1s = ctx.enter_context(tc.tile_pool(name="w1s", bufs=3))
        w2s = ctx.enter_context(tc.tile_pool(name="w2s", bufs=3))
        attp = ctx.enter_context(tc.tile_pool(name="attp", bufs=2))
        vec = ctx.enter_context(tc.tile_pool(name="vec", bufs=4))
        rrp = ctx.enter_context(tc.tile_pool(name="rrp", bufs=2))
        jk = ctx.enter_context(tc.tile_pool(name="jk", bufs=3))
        scp = ctx.enter_context(tc.tile_pool(name="scp", bufs=2, space="PSUM"))
        pAV = ctx.enter_context(tc.tile_pool(name="pAV", bufs=2, space="PSUM"))
        pP = ctx.enter_context(tc.tile_pool(name="pP", bufs=2, space="PSUM"))

        # ------------- persistent tiles -------------
        qkw_sb = pers.tile([P, 12, CCN, P], H, name="qkw_sb")
        vw_sb = pers.tile([P, CCN, VA], H, name="vw_sb")
        wo_sb = pers.tile([P, CCN, CCN, P], H, name="wo_sb")
        bqk_sb = pers.tile([P, 12], FP, name="bqk_sb")
        bo_sb = pers.tile([P, CCN], FP, name="bo_sb")
        b1_sb = pers.tile([P, HCN], FP, name="b1_sb")
        b2_sb = pers.tile([P, CCN], FP, name="b2_sb")
        vb_sb = pers.tile([1, VA], H, name="vb_sb")
        ident16 = pers.tile([P, P], H, name="ident16")
        ident32 = pers.tile([P, P], FP, name="ident32")
        ones_sb = pers.tile([P, P], H, name="ones_sb")
        z_sb = pers.tile([P, TCH, C], FP, name="z_sb")
        stat = pers.tile([P, 8, TCH], FP, name="stat")
        eps_col = pers.tile([P, 1], FP, name="eps_col")
        # attention K/V (feature-major K, token-major V), local + remote
        k_loc = pers.tile([P, CCN, TL], H, name="k_loc")
        k_rem = pers.tile([P, CCN, TL], H, name="k_rem")
        v_loc = pers.tile([P, TCH, VA], H, name="v_loc")
        v_rem = pers.tile([P, TCH, VA], H, name="v_rem")
        tb = pers.tile([64, CCN, TL], H, name="tb")
        # Anderson state: residual history + cached Gram matrix
        fh = pers.tile([P, MH, TCH, C], H, name="fh")
        m_sb = pers.tile([P, TCH, MH, MH], FP, name="m_sb")
        a_sb = pers.tile([P, TCH, 4, 5], FP, name="a_sb")   # [G | b] rows
        alt = pers.tile([P, TCH, 4], FP, name="alt")        # solution x
        coef = pers.tile([P, TCH, MH], FP, name="coef")
        rin = pers.tile([P, TCH, 4], FP, name="rin")        # pivots' recips
        sc1 = pers.tile([P, TCH], FP, name="sc1")
        sc3 = pers.tile([P, TCH], FP, name="sc3")

        nc.sync.dma_start(qkw_sb[:], qkw_d[:])
        nc.sync.dma_start(vw_sb[:], vw_d[:])
        nc.sync.dma_start(wo_sb[:], wo_d[:])
        nc.sync.dma_start(bqk_sb[:], bqk_d[:])
        nc.sync.dma_start(bo_sb[:], bo_d[:])
        nc.sync.dma_start(b1_sb[:], b1_d[:])
        nc.sync.dma_start(b2_sb[:], b2_d[:])
        nc.sync.dma_start(vb_sb[:], vb_d[:])
        make_identity(nc, ident16[:])
        make_identity(nc, ident32[:])
        nc.vector.memset(ones_sb[:], 1.0)
        nc.vector.memset(eps_col[:], LN_EPS)

        TT = nc.vector.tensor_tensor
        TS = nc.vector.tensor_scalar
        STT = nc.vector.scalar_tensor_tensor

        def layernorm(src, dst, sc):
            """token-major LN without weight/bias (folded into next matmul).
            src/dst: [P, TCH, C] fp16; stats use stat cols 4*sc..4*sc+3.
            sums on DVE, square-sums on scalar (parallel engines);
            rsqrt via ln+exp (stays in the exp table set)."""
            i0, i1, i2, i3 = 4 * sc, 4 * sc + 1, 4 * sc + 2, 4 * sc + 3
            for t in range(TCH):
                j1 = jk.tile([P, C], H, name="jk")
                with nc.allow_low_precision(reason="junk out"):
                    TS(out=j1[:], in0=src[:, t], scalar1=1.0, scalar2=0.0,
                       op0=OP.mult, op1=OP.add,
                       accum_out=stat[:, i0, t:t + 1])
                j2 = jk.tile([P, C], H, name="jk")
                nc.scalar.activation(j2[:], src[:, t], AF.Square,
                                     accum_out=stat[:, i1, t:t + 1])
            TS(out=stat[:, i2], in0=stat[:, i0], scalar1=1.0 / C,
               scalar2=None, op0=OP.mult)                       # mu
            TT(out=stat[:, i0], in0=stat[:, i2], in1=stat[:, i2],
               op=OP.mult)                                      # mu^2
            STT(out=stat[:, i1], in0=stat[:, i1], scalar=1.0 / C,
                in1=stat[:, i0], op0=OP.mult, op1=OP.subtract)  # var
            nc.scalar.activation(stat[:, i0], stat[:, i1], AF.Ln,
                                 bias=eps_col[:])               # ln(var+eps)
            nc.scalar.activation(stat[:, i3], stat[:, i0], AF.Exp,
                                 scale=-0.5)                    # rsqrt
            for t in range(TCH):
                with nc.allow_low_precision(reason="fp16 ln out"):
                    TS(out=dst[:, t], in0=src[:, t],
                       scalar1=stat[:, i2, t:t + 1],
                       scalar2=stat[:, i3, t:t + 1],
                       op0=OP.subtract, op1=OP.mult)

        def transpose_fm(src_tok, dst_fm):
            """[P, TCH, C] fp16 token-major -> [P, CCN, TL] feature-major."""
            for cc in range(CCN):
                ptr = pP.tile([P, TCH, P], H, name="pP")
                for t in range(TCH):
                    nc.tensor.transpose(
                        ptr[:, t], src_tok[:, t, cc * P:(cc + 1) * P],
                        ident16[:])
                nc.scalar.copy(dst_fm[:, cc], ptr[:])

        def transpose_tok(src_fm, dst_tok):
            """[P, CCN, TL] fp16 feature-major -> [P, TCH, C] token-major."""
            for t in range(TCH):
                ptr = pP.tile([P, CCN, P], H, name="pP")
                for cc in range(CCN):
                    nc.tensor.transpose(
                        ptr[:, cc], src_fm[:, cc, t * P:(t + 1) * P],
                        ident16[:])
                nc.scalar.copy(dst_tok[:, t], ptr[:])

        def keepalive(dep_ap):
            """tiny fp32 transpose reading dep_ap ([P, n<=128]): keeps the PE
            HAM window busy during vector-engine-heavy phases."""
            n = dep_ap.shape[-1]
            jp = pP.tile([P, P], FP, name="pP")
            nc.tensor.transpose(jp[0:n, :], dep_ap, ident32[:])

        ut_tiles = {}

        def load_u(it):
            if it < ni and it not in ut_tiles:
                t_ = uitp.tile([P, TCH, C], H, name="uitp")
                nc.sync.dma_start(
                    t_[:], uit_d[it].rearrange("(t p) c -> p t c", p=P))
                ut_tiles[it] = t_

        load_u(0)

        for it in range(ni):
            Kn = min(it, 4)
            s_new = it % MH
            prev = [(it - Kn + k) % MH for k in range(Kn)]  # oldest..newest

            # ---- A: zctx = z + (u + 0.1 emb_it); LN1 -> x1 (fm) ----
            ut = ut_tiles.pop(it)
            zctx = big16.tile([P, TCH, C], H, name="big16")
            if it == 0:
                nc.vector.tensor_copy(zctx[:], ut[:])
            else:
                with nc.allow_low_precision(reason="fp16 zctx"):
                    TT(out=zctx[:], in0=z_sb[:], in1=ut[:], op=OP.add)
            x1t = big16.tile([P, TCH, C], H, name="big16")
            layernorm(zctx, x1t, 0)
            x1 = fm16.tile([P, CCN, TL], H, name="fm16")
            transpose_fm(x1t, x1)

            # ---- B: K projection -> local tile + AllReduce ----
            for oc in range(CCN):
                pk = pP.tile([P, TL], FP, name="pP")
                for cc in range(CCN):
                    nc.tensor.matmul(pk[:], qkw_sb[:, 6 + oc, cc], x1[:, cc],
                                     start=(cc == 0), stop=(cc == CCN - 1))
                nc.scalar.activation(k_loc[:, oc], pk[:], AF.Identity,
                                     bias=bqk_sb[:, 6 + oc:7 + oc])
            nc.sync.dma_start(kcc[:].rearrange("(cc p) t -> p cc t", p=P),
                              k_loc[:])
            nc.gpsimd.collective_compute(
                "AllReduce", OP.add, replica_groups=GROUPS,
                ins=[kcc[:]], outs=[kred[:]])

            # ---- C: V projection (token-major) -> AllReduce ----
            for t in range(TCH):
                pva = pAV.tile([P, TL], FP, name="pAV")
                pvb = pP.tile([P, VA - TL], FP, name="pP")
                for cc in range(CCN):
                    nc.tensor.matmul(pva[:], x1[:, cc, t * P:(t + 1) * P],
                                     vw_sb[:, cc, 0:TL],
                                     start=(cc == 0), stop=False)
                    nc.tensor.matmul(pvb[:], x1[:, cc, t * P:(t + 1) * P],
                                     vw_sb[:, cc, TL:VA],
                                     start=(cc == 0), stop=False)
                nc.tensor.matmul(pva[:], ones_sb[0:1, :], vb_sb[:, 0:TL],
                                 start=False, stop=True)
                nc.tensor.matmul(pvb[:], ones_sb[0:1, :], vb_sb[:, TL:VA],
                                 start=False, stop=True)
                nc.scalar.copy(v_loc[:, t, 0:TL], pva[:])
                nc.scalar.copy(v_loc[:, t, TL:VA], pvb[:])
            nc.sync.dma_start(vcc[:].rearrange("(t p) c -> p t c", p=P),
                              v_loc[:])
            nc.gpsimd.collective_compute(
                "AllReduce", OP.add, replica_groups=GROUPS,
                ins=[vcc[:]], outs=[vred[:]])

            # ---- D: Q projection (overlaps collectives) ----
            q_sb = qp.tile([P, CCN, TL], H, name="q_sb")
            for oc in range(CCN):
                pq = pP.tile([P, TL], FP, name="pP")
                for cc in range(CCN):
                    nc.tensor.matmul(pq[:], qkw_sb[:, oc, cc], x1[:, cc],
                                     start=(cc == 0), stop=(cc == CCN - 1))
                nc.scalar.activation(q_sb[:, oc], pq[:], AF.Identity,
                                     bias=bqk_sb[:, oc:oc + 1])

            # ---- E: remote K/V = allreduce - local (in place) ----
            nc.sync.dma_start(k_rem[:],
                              kred[:].rearrange("(cc p) t -> p cc t", p=P))
            with nc.allow_low_precision(reason="fp16 k rem"):
                TT(out=k_rem[:], in0=k_rem[:], in1=k_loc[:], op=OP.subtract)
            nc.sync.dma_start(v_rem[:],
                              vred[:].rearrange("(t p) c -> p t c", p=P))
            with nc.allow_low_precision(reason="fp16 v rem"):
                TT(out=v_rem[:], in0=v_rem[:], in1=v_loc[:], op=OP.subtract)

            # prefetch next iteration's u while attention runs
            load_u(it + 1)

            # ---- F: attention (local kc first, then remote) ----
            out_fm = ofm.tile([P, CCN, TL], H, name="ofm")
            for hp in range(HPN):
                for half in range(2):          # 0: head hp, 1: head hp+6
                    ksrc = slice(64 * half, 64 * (half + 1))
                    hoff = (hp + 6 * half) * VW
                    pav = pAV.tile([64, TL], FP, name="pAV")
                    psm = pAV.tile([64, TL], FP, name="pAV")
                    for g in range(4):         # kc pairs: local 0-1, remote 2-3
                        ktile = k_loc if g < 2 else k_rem
                        vtile = v_loc if g < 2 else v_rem
                        sc2b = scp.tile([P, 2, TL], FP, name="scp")
                        for j in range(2):
                            kj = (g % 2) * 2 + j
                            ks = slice(kj * P, (kj + 1) * P)
                            nc.tensor.matmul(sc2b[:, j], ktile[ksrc, hp, ks],
                                             q_sb[ksrc, hp],
                                             start=True, stop=True)
                        att = attp.tile([P, 2, TL], H, name="attp")
                        nc.scalar.activation(att[:], sc2b[:], AF.Exp,
                                             scale=0.125)
                        for j in range(2):
                            kc = g * 2 + j
                            kj = (g % 2) * 2 + j
                            nc.tensor.matmul(pav[:],
                                             vtile[:, kj, hoff:hoff + VW],
                                             att[:, j], start=(kc == 0),
                                             stop=(kc == KCN - 1))
                            nc.tensor.matmul(psm[:], ones_sb[:, 0:64],
                                             att[:, j], start=(kc == 0),
                                             stop=(kc == KCN - 1))
                    ra = rrp.tile([64, TL], FP, name="rrp")
                    nc.vector.reciprocal_approx_fast(ra[:], psm[:])
                    with nc.allow_low_precision(reason="fp16 attn"):
                        if half == 0:
                            TT(out=out_fm[0:64, hp], in0=pav[0:64, :],
                               in1=ra[:], op=OP.mult)
                        else:
                            TT(out=tb[:, hp], in0=pav[0:64, :], in1=ra[:],
                               op=OP.mult)
            nc.sync.dma_start(out_fm[64:128, :, :], tb[:])

            # ---- G: output projection -> attn (feature-major fp16) ----
            attn_fm = atp.tile([P, CCN, TL], H, name="atp")
            for oc in range(CCN):
                pp_ = pP.tile([P, TL], FP, name="pP")
                for ci in range(CCN):
                    nc.tensor.matmul(pp_[:], wo_sb[:, oc, ci], out_fm[:, ci],
                                     start=(ci == 0), stop=(ci == CCN - 1))
                nc.scalar.activation(attn_fm[:, oc], pp_[:], AF.Identity,
                                     bias=bo_sb[:, oc:oc + 1])

            # ---- H: za = z + attn (token-major); LN2 -> x2 (fm) ----
            attn_tok = tokp.tile([P, TCH, C], H, name="tokp")
            transpose_tok(attn_fm, attn_tok)
            if it == 0:
                za = attn_tok
            else:
                za = big16.tile([P, TCH, C], H, name="big16")
                with nc.allow_low_precision(reason="fp16 za"):
                    TT(out=za[:], in0=z_sb[:], in1=attn_tok[:], op=OP.add)
            x2t = big16.tile([P, TCH, C], H, name="big16")
            layernorm(za, x2t, 1)
            x2 = fm16.tile([P, CCN, TL], H, name="fm16")
            transpose_fm(x2t, x2)

            # ---- I: MLP (streamed weights, streaming W2 accumulation) ----
            p2t1 = scp.tile([P, 2, TL], FP, name="scp")
            p2t2 = scp.tile([P, 2, TL], FP, name="scp")
            p2s = [p2t1[:, 0], p2t1[:, 1], p2t2[:, 0], p2t2[:, 1],
                   pAV.tile([P, TL], FP, name="pAV"),
                   pAV.tile([P, TL], FP, name="pAV")]
            for hi in range(HCN):
                w1t = w1s.tile([P, CCN, P], H, name="w1s")
                nc.sync.dma_start(w1t[:], w1_d[hi])
                ph = pP.tile([P, TL], FP, name="pP")
                for cc in range(CCN):
                    nc.tensor.matmul(ph[:], w1t[:, cc], x2[:, cc],
                                     start=(cc == 0), stop=(cc == CCN - 1))
                gt = gp.tile([P, TL], H, name="gp")
                nc.scalar.activation(gt[:], ph[:], AF.Gelu,
                                     bias=b1_sb[:, hi:hi + 1])
                w2t = w2s.tile([P, CCN, P], H, name="w2s")
                nc.sync.dma_start(w2t[:], w2_d[hi])
                for oc in range(CCN):
                    nc.tensor.matmul(p2s[oc], w2t[:, oc], gt[:],
                                     start=(hi == 0), stop=(hi == HCN - 1))
            res_fm = ofm.tile([P, CCN, TL], H, name="ofm")
            for oc in range(CCN):
                with nc.allow_low_precision(reason="fp16 res"):
                    STT(out=res_fm[:, oc], in0=p2s[oc],
                        scalar=b2_sb[:, oc:oc + 1], in1=attn_fm[:, oc],
                        op0=OP.add, op1=OP.add)

            # ---- J: res -> token-major, straight into the history slot ----
            resq = fh[:, s_new]
            transpose_tok(res_fm, resq)

            # ---- K: Anderson update with cached Gram matrix ----
            # new dots: <F_k, res> for active k plus <res, res>
            for k in prev + [s_new]:
                for t in range(TCH):
                    j1 = jk.tile([P, C], H, name="jk")
                    with nc.allow_low_precision(reason="junk out"):
                        STT(out=j1[:], in0=fh[:, k, t], scalar=1.0,
                            in1=resq[:, t], op0=OP.mult, op1=OP.mult,
                            accum_out=m_sb[:, t, k, s_new:s_new + 1])
            # mirror new column into the row
            nc.vector.tensor_copy(m_sb[:, :, s_new, :], m_sb[:, :, :, s_new])
            keepalive(sc1[:])

            if Kn == 0:
                nc.vector.tensor_copy(z_sb[:], resq[:])
            else:
                # assemble [G | b] into a_sb rows; prev is a contiguous
                # ascending slot range for ni <= 6
                lo = prev[0]
                assert prev == list(range(lo, lo + Kn)), "slot wrap"
                n = s_new
                pa = slice(lo, lo + Kn)
                nnb = m_sb[:, :, n, n:n + 1]
                for ai, a in enumerate(prev):
                    # G[a,:] = M[a,pa] - M[a,n] - M[n,pa] + M[n,n]
                    TT(out=a_sb[:, :, ai, 0:Kn], in0=m_sb[:, :, a, pa],
                       in1=m_sb[:, :, a, n:n + 1].broadcast_to([P, TCH, Kn]),
                       op=OP.subtract)
                    TT(out=a_sb[:, :, ai, 0:Kn], in0=a_sb[:, :, ai, 0:Kn],
                       in1=m_sb[:, :, n, pa], op=OP.subtract)
                    TT(out=a_sb[:, :, ai, 0:Kn], in0=a_sb[:, :, ai, 0:Kn],
                       in1=nnb.broadcast_to([P, TCH, Kn]), op=OP.add)
                    # b[a] = M[a,n] - M[n,n]
                    TT(out=a_sb[:, :, ai, Kn], in0=m_sb[:, :, a, n],
                       in1=m_sb[:, :, n, n], op=OP.subtract)
                for ai in range(Kn):
                    TS(out=a_sb[:, :, ai, ai:ai + 1],
                       in0=a_sb[:, :, ai, ai:ai + 1],
                       scalar1=1e-6, scalar2=None, op0=OP.add)
                keepalive(a_sb[:, 0, 0])

                # forward elimination on rows [G | b]
                W = Kn + 1
                for i in range(Kn):
                    nc.vector.reciprocal(rin[:, :, i], a_sb[:, :, i, i])
                    for j in range(i + 1, Kn):
                        TT(out=sc1[:], in0=a_sb[:, :, j, i],
                           in1=rin[:, :, i], op=OP.mult)
                        t1 = vec.tile([P, TCH, 5], FP, name="vrow")
                        TT(out=t1[:, :, 0:W - i], in0=a_sb[:, :, i, i:W],
                           in1=sc1[:, :, None].broadcast_to([P, TCH, W - i]),
                           op=OP.mult)
                        TT(out=a_sb[:, :, j, i:W], in0=a_sb[:, :, j, i:W],
                           in1=t1[:, :, 0:W - i], op=OP.subtract)
                    if i == 1:
                        keepalive(a_sb[:, 0, 0])
                # back substitution
                for i in range(Kn - 1, -1, -1):
                    nc.vector.tensor_copy(sc3[:], a_sb[:, :, i, Kn])
                    for j in range(i + 1, Kn):
                        TT(out=sc1[:], in0=a_sb[:, :, i, j], in1=alt[:, :, j],
                           op=OP.mult)
                        TT(out=sc3[:], in0=sc3[:], in1=sc1[:], op=OP.subtract)
                    TT(out=alt[:, :, i], in0=sc3[:], in1=rin[:, :, i],
                       op=OP.mult)
                keepalive(alt[:, 0])

                # coef col 0 = 1 + sum(alpha); cols 1..Kn = -alpha
                if Kn == 1:
                    TS(out=coef[:, :, 0], in0=alt[:, :, 0], scalar1=1.0,
                       scalar2=None, op0=OP.add)
                else:
                    nc.vector.tensor_copy(sc1[:], alt[:, :, 0])
                    for k in range(1, Kn):
                        TT(out=sc1[:], in0=sc1[:], in1=alt[:, :, k],
                           op=OP.add)
                    TS(out=coef[:, :, 0], in0=sc1[:], scalar1=1.0,
                       scalar2=None, op0=OP.add)
                TS(out=coef[:, :, 1:1 + Kn], in0=alt[:, :, 0:Kn],
                   scalar1=-1.0, scalar2=None, op0=OP.mult)

                # z += c0*res + sum_k c_{k+1}*F_k
                for t in range(TCH):
                    STT(out=z_sb[:, t], in0=resq[:, t],
                        scalar=coef[:, t, 0:1], in1=z_sb[:, t],
                        op0=OP.mult, op1=OP.add)
                    for k in range(Kn):
                        STT(out=z_sb[:, t], in0=fh[:, prev[k], t],
                            scalar=coef[:, t, k + 1:k + 2], in1=z_sb[:, t],
                            op0=OP.mult, op1=OP.add)
                keepalive(coef[:, 0])

        for t in range(TCH):
            nc.sync.dma_start(zo_d[t * P:(t + 1) * P, :], z_sb[:, t])

        ctx.close()

    nc.finalize()
    return nc


def _host_pack(inputs, num_iters):
    f32 = np.float32
    f16 = np.float16
    ipw = np.ascontiguousarray(inputs["in_proj_w"], f32)
    ipb = np.ascontiguousarray(inputs["in_proj_b"], f32)
    opw = np.ascontiguousarray(inputs["out_proj_w"], f32)
    opb = np.ascontiguousarray(inputs["out_proj_b"], f32)
    w1 = np.ascontiguousarray(inputs["mlp_w1"], f32)
    b1 = np.ascontiguousarray(inputs["mlp_b1"], f32)
    w2 = np.ascontiguousarray(inputs["mlp_w2"], f32)
    b2 = np.ascontiguousarray(inputs["mlp_b2"], f32)
    emb = np.ascontiguousarray(inputs["iter_emb"], f32)
    ln1_w = np.asarray(inputs["ln1_w"], f32)
    ln1_b = np.asarray(inputs["ln1_b"], f32)
    ln2_w = np.asarray(inputs["ln2_w"], f32)
    ln2_b = np.asarray(inputs["ln2_b"], f32)

    # fold LN1 into in_proj, LN2 into mlp_w1
    ipw_f = ipw * ln1_w[None, :]
    ipb_f = ipb + ipw @ ln1_b
    w1_f = w1 * ln2_w[None, :]
    b1_f = b1 + w1 @ ln2_b

    # head permutation: attn chunk j holds head j (rows 0:64), head j+6
    # (rows 64:128)
    hperm = np.zeros(C, np.int64)
    for j in range(HPN):
        hperm[j * P:j * P + 64] = np.arange(j * 64, (j + 1) * 64)
        hperm[j * P + 64:(j + 1) * P] = np.arange((j + 6) * 64, (j + 7) * 64)

    qw = ipw_f[0:C][hperm]
    kw = ipw_f[C:2 * C][hperm]
    vw = ipw_f[2 * C:3 * C]
    qb = ipb_f[0:C][hperm]
    kb = ipb_f[C:2 * C][hperm]
    vb = ipb_f[2 * C:3 * C]

    # qkw_pack [P(c within chunk), 12, CCN, P(m)]: chunks 0..5 q, 6..11 k
    qkw = np.concatenate([qw.reshape(CCN, P, C), kw.reshape(CCN, P, C)], 0)
    qkw_pack = np.ascontiguousarray(
        qkw.reshape(12, P, CCN, P).transpose(3, 0, 2, 1).astype(f16))

    # vw_pack [P(c), CCN, VA] (plain v feature order)
    vw_aug = vw.T.astype(f32)
    vb_aug = vb.astype(f32)
    vw_pack = np.ascontiguousarray(
        vw_aug.reshape(CCN, P, VA).transpose(1, 0, 2).astype(f16))

    # wo_pack [P(c-attnfeat), oc, ci, P(m)] (columns permuted by hperm)
    opw_p = opw[:, hperm]
    wo_pack = np.ascontiguousarray(
        opw_p.reshape(CCN, P, CCN, P).transpose(3, 0, 2, 1).astype(f16))

    # w1_pack [hi, P(c), cc, P(m)]
    w1_pack = np.ascontiguousarray(
        w1_f.reshape(HCN, P, CCN, P).transpose(0, 3, 2, 1).astype(f16))

    # w2_pack [hi, P(hid c), oc, P(m)]
    w2_pack = np.ascontiguousarray(
        w2.reshape(CCN, P, HCN, P).transpose(2, 3, 0, 1).astype(f16))

    bqk_cols = np.ascontiguousarray(
        np.concatenate([qb, kb]).reshape(12, P).T.astype(f32))
    bo_cols = np.ascontiguousarray(opb.reshape(CCN, P).T.astype(f32))
    b1_cols = np.ascontiguousarray(b1_f.reshape(HCN, P).T.astype(f32))
    b2_cols = np.ascontiguousarray(b2.reshape(CCN, P).T.astype(f32))

    rows = [min(i, emb.shape[0] - 1) for i in range(num_iters)]
    u = np.ascontiguousarray(inputs["u"], f32)

    shared = dict(
        qkw_pack=qkw_pack, vw_pack=vw_pack, wo_pack=wo_pack, w1_pack=w1_pack,
        w2_pack=w2_pack, vb_aug=vb_aug.reshape(1, VA).astype(f16),
        bqk_cols=bqk_cols, bo_cols=bo_cols, b1_cols=b1_cols, b2_cols=b2_cols)
    in_maps = []
    for core in range(NCORES):
        b, h = core // 2, core % 2
        m = dict(shared)
        useg = u[b, h * TL:(h + 1) * TL, :]
        u_it = useg[None] + 0.1 * emb[rows][:, None, :]
        m["u_it"] = np.ascontiguousarray(u_it.astype(f16))
        in_maps.append(m)
    return in_maps


def run_device(inputs, num_iters=None, trace=False):
    from concourse.bass_utils import run_bass_kernel_spmd
    ni = int(inputs.get("num_iters", 6)) if num_iters is None else num_iters
    u = inputs["u"]
    B, T, _ = u.shape
    if ni == 0:
        return np.zeros((B, T, C), np.float32), None
    if ni not in _CACHE:
        _CACHE[ni] = _build(ni)
    nc = _CACHE[ni]
    in_maps = _host_pack(inputs, ni)
    r = run_bass_kernel_spmd(nc, in_maps, list(range(NCORES)), trace=trace)
    out = np.empty((B, T, C), np.float32)
    for core in range(NCORES):
        b, h = core // 2, core % 2
        out[b, h * TL:(h + 1) * TL, :] = r.results[core]["z_out"]
    return out, r


def kernel(**inputs):
    out, _ = run_device(inputs)
    return out.astype(np.float32)
